# revision 8
# baseline (speedup 1.0000x reference)
"""Batched dynamic-filter cross-correlation on 8 Trainium2 NeuronCores.

Each sample b of x[128, 384, 384, 1] is VALID-correlated with its own
8x8 filter k[b] -> out[128, 377, 377, 1].

Strategy (pure data parallel, batch sharded 16 samples/core): the row
taps (p) contract on the TensorE partition dim via banded-Toeplitz
stationary matrices; the column taps (q) use a 2-parallel fast-FIR
(Karatsuba) decomposition to cut TensorE streaming work to 3/4:
  out[:, even] and out[:, odd] are recovered from three 4-tap
  half-rate sub-correlations P0 = H0*X0, P2 = H1*X1,
  P1 = (H0+H1)*(X0+X1), where X0/X1 are even/odd input columns
  (stride-2 access patterns, no copies) and H0/H1 even/odd taps of the
  q-reversed filter. Per 121-row output block this needs 12
  PSUM-accumulating matmuls of N=189 instead of 8 of N=378.
  out_even = P1 - P0 - P2, out_odd = shift(P0) + P2 are recombined by
  the vector/pool engines directly into the bf16 output tile.
The 14 leftover output rows of 4 samples are packed into one
block-diagonal 8-tap matmul group (K=4*21, M=4*14). All HBM tensors
are bf16 (accumulation stays fp32 in PSUM; host upcasts the output).
"""

import numpy as np
import ml_dtypes

BF16 = ml_dtypes.bfloat16

B, H, W = 128, 384, 384
KH, KW = 8, 8
HO, WO = H - KH + 1, W - KW + 1          # 377, 377
N_CORES = 8
SPC = B // N_CORES                        # 16 samples per core

MAIN_BLOCKS = [(0, 121, 128), (121, 121, 128), (242, 121, 128)]
TB, TM, TK = 363, 14, 21                  # tail rows: out 363..376, in 363..383
GS = 4                                    # tail-group size (samples per group)
NO2 = WO + 1                              # 378: tail moving width
XW = 386                                  # tail x tile width (q=7 reads col 384)
NP = 189                                  # half-rate sub-conv output cols (m=3..191)
XH = 192                                  # half-rate input length

_cache = {}


def _build_program():
    import concourse.mybir as mybir
    import concourse.tile as tile
    from concourse import bacc

    bf16 = mybir.dt.bfloat16
    f32 = mybir.dt.float32
    nc = bacc.Bacc(None, target_bir_lowering=False)
    x_d = nc.dram_tensor("x", [SPC, H, W], bf16, kind="ExternalInput")
    b_d = nc.dram_tensor("bands", [SPC, 128, 12, 121], bf16, kind="ExternalInput")
    t_d = nc.dram_tensor(
        "tailbands", [SPC // GS, GS * TK, KW, GS * TM], bf16, kind="ExternalInput"
    )
    o_d = nc.dram_tensor("out", [SPC, HO, WO], bf16, kind="ExternalOutput")

    with tile.TileContext(nc) as tc:
        with (
            tc.tile_pool(name="xp", bufs=6) as xp,
            tc.tile_pool(name="xsp", bufs=6) as xsp,
            tc.tile_pool(name="bp", bufs=3) as bp,
            tc.tile_pool(name="tbp", bufs=2) as tbp,
            tc.tile_pool(name="txp", bufs=2) as txp,
            tc.tile_pool(name="pa", bufs=2, space="PSUM") as pa,
            tc.tile_pool(name="pb", bufs=2, space="PSUM") as pb,
            tc.tile_pool(name="pc", bufs=2, space="PSUM") as pc,
            tc.tile_pool(name="pt", bufs=2, space="PSUM") as pt,
            tc.tile_pool(name="tp", bufs=4) as tp,
            tc.tile_pool(name="op", bufs=6) as op,
        ):
            for g in range(SPC // GS):
                for j in range(GS):
                    s = g * GS + j
                    bt = bp.tile([128, 12, 121], bf16)
                    nc.scalar.dma_start(out=bt[:], in_=b_d[s])
                    for obase, M, K in MAIN_BLOCKS:
                        xt = xp.tile([128, W], bf16)
                        nc.gpsimd.dma_start(
                            out=xt[:K, :], in_=x_d[s, obase : obase + K, :]
                        )
                        # X0+X1 pre-sum for the P1 sub-conv
                        xs = xsp.tile([128, XH], bf16)
                        nc.vector.tensor_add(
                            out=xs[:K, :], in0=xt[:K, 0:W:2], in1=xt[:K, 1:W:2]
                        )
                        ps0 = pa.tile([128, 512], f32)
                        ps2 = pb.tile([128, 512], f32)
                        ps1 = pc.tile([128, 512], f32)
                        for u in range(4):
                            st = 2 * (3 - u)
                            nc.tensor.matmul(
                                ps0[:M, :NP],
                                bt[:K, u, :M],
                                xt[:K, st : st + 2 * NP - 1 : 2],
                                start=(u == 0),
                                stop=(u == 3),
                            )
                        for u in range(4):
                            st = 2 * (3 - u) + 1
                            nc.tensor.matmul(
                                ps2[:M, :NP],
                                bt[:K, 4 + u, :M],
                                xt[:K, st : st + 2 * NP - 1 : 2],
                                start=(u == 0),
                                stop=(u == 3),
                            )
                        for u in range(4):
                            st = 3 - u
                            nc.tensor.matmul(
                                ps1[:M, :NP],
                                bt[:K, 8 + u, :M],
                                xs[:K, st : st + NP],
                                start=(u == 0),
                                stop=(u == 3),
                            )
                        ot = op.tile([128, WO], bf16)
                        # out_even[t] = P1[t]-P0[t]-P2[t]; out_odd[t] = P0[t+1]+P2[t]
                        # (EW ops may read only one PSUM operand: stage P0 in SBUF)
                        c0 = tp.tile([128, NP], f32)
                        nc.scalar.copy(out=c0[:M, :], in_=ps0[:M, :NP])
                        c2 = tp.tile([128, NP], f32)
                        nc.scalar.copy(out=c2[:M, :], in_=ps2[:M, :NP])
                        t0 = tp.tile([128, NP], f32)
                        nc.vector.tensor_sub(
                            out=t0[:M, :], in0=ps1[:M, :NP], in1=c0[:M, :]
                        )
                        nc.vector.tensor_sub(
                            out=ot[:M, 0 : WO : 2], in0=t0[:M, :], in1=c2[:M, :]
                        )
                        nc.gpsimd.tensor_add(
                            out=ot[:M, 1 : WO : 2],
                            in0=c0[:M, 1:NP],
                            in1=c2[:M, 0 : NP - 1],
                        )
                        nc.sync.dma_start(
                            out=o_d[s, obase : obase + M, :], in_=ot[:M, :]
                        )
                # tail group: GS samples' last 14 rows, block-diagonal matmul
                tt = tbp.tile([GS * TK, KW, GS * TM], bf16)
                nc.scalar.dma_start(out=tt[:], in_=t_d[g])
                xtt = txp.tile([128, XW], bf16)
                nc.vector.memset(xtt[:, W:XW], 0.0)
                for j in range(GS):
                    nc.gpsimd.dma_start(
                        out=xtt[TK * j : TK * j + TK, :W],
                        in_=x_d[g * GS + j, TB : TB + TK, :],
                    )
                ps = pt.tile([128, 512], f32)
                for q in range(KW):
                    nc.tensor.matmul(
                        ps[: GS * TM, :NO2],
                        tt[: GS * TK, q, : GS * TM],
                        xtt[: GS * TK, q : q + NO2],
                        start=(q == 0),
                        stop=(q == KW - 1),
                    )
                ot = op.tile([128, WO], bf16)
                nc.scalar.copy(out=ot[: GS * TM, :], in_=ps[: GS * TM, :WO])
                for j in range(GS):
                    nc.sync.dma_start(
                        out=o_d[g * GS + j, TB : TB + TM, :],
                        in_=ot[TM * j : TM * j + TM, :],
                    )

    nc.compile()
    return nc


def _build_runner():
    """Build nc + a persistent jitted PJRT callable (compiles once)."""
    import jax
    from jax.sharding import Mesh, PartitionSpec
    from jax.experimental.shard_map import shard_map
    import concourse.mybir as mybir
    from concourse import bass2jax

    nc = _build_program()
    bass2jax.install_neuronx_cc_hook()

    partition_name = nc.partition_id_tensor.name if nc.partition_id_tensor else None

    in_names, out_names, out_avals, zero_shapes = [], [], [], []
    for alloc in nc.m.functions[0].allocations:
        if not isinstance(alloc, mybir.MemoryLocationSet):
            continue
        name = alloc.memorylocations[0].name
        if alloc.kind == "ExternalInput":
            if name != partition_name:
                in_names.append(name)
        elif alloc.kind == "ExternalOutput":
            shape = tuple(alloc.tensor_shape)
            dtype = mybir.dt.np(alloc.dtype)
            out_names.append(name)
            out_avals.append(jax.core.ShapedArray(shape, dtype))
            zero_shapes.append((shape, dtype))
    n_params = len(in_names)
    n_outs = len(out_avals)
    all_in_names = list(in_names) + list(out_names)
    if partition_name is not None:
        all_in_names.append(partition_name)

    def _body(*args):
        operands = list(args)
        if partition_name is not None:
            operands.append(bass2jax.partition_id_tensor())
        outs = bass2jax._bass_exec_p.bind(
            *operands,
            out_avals=tuple(out_avals),
            in_names=tuple(all_in_names),
            out_names=tuple(out_names),
            lowering_input_output_aliases=(),
            sim_require_finite=True,
            sim_require_nnan=True,
            nc=nc,
        )
        return tuple(outs)

    devices = jax.devices()[:N_CORES]
    mesh = Mesh(np.asarray(devices), ("core",))
    in_specs = (PartitionSpec("core"),) * (n_params + n_outs)
    out_specs = (PartitionSpec("core"),) * n_outs
    sharded = jax.jit(
        shard_map(
            _body, mesh=mesh, in_specs=in_specs, out_specs=out_specs, check_rep=False
        ),
        keep_unused=True,
    )

    from jax.sharding import NamedSharding

    zero_sharding = NamedSharding(mesh, PartitionSpec("core"))
    dev_zeros = [
        jax.device_put(np.zeros((N_CORES * s[0], *s[1:]), d), zero_sharding)
        for (s, d) in zero_shapes
    ]

    def run(in_maps):
        concat_in = [
            np.concatenate([np.asarray(m[name]) for m in in_maps], axis=0)
            for name in in_names
        ]
        out_arrs = sharded(*concat_in, *dev_zeros)
        return [
            {
                name: np.asarray(out_arrs[i]).reshape(
                    N_CORES, *out_avals[i].shape
                )[c]
                for i, name in enumerate(out_names)
            }
            for c in range(N_CORES)
        ]

    return nc, run


def _build_bands(k2):
    """k2: [B, 8, 8] fp32 -> Karatsuba bands [B, 128, 12, 121] bf16.

    h = q-reversed filter; H0/H1 = even/odd taps (4 each); planes
    0-3: Toeplitz bands of H0, 4-7: H1, 8-11: H0+H1.
    bands[b, m+p, plane(u), m] = Hx[b, p, u].
    """
    h = k2[:, :, ::-1]
    H0 = h[:, :, 0::2].astype(BF16)
    H1 = h[:, :, 1::2].astype(BF16)
    HS = (h[:, :, 0::2] + h[:, :, 1::2]).astype(BF16)
    bands = np.zeros((k2.shape[0], 128, 12, 121), BF16)
    m = np.arange(121)
    for p in range(KH):
        for u in range(4):
            bands[:, m + p, u, m] = H0[:, p, u][:, None]
            bands[:, m + p, 4 + u, m] = H1[:, p, u][:, None]
            bands[:, m + p, 8 + u, m] = HS[:, p, u][:, None]
    return bands


def _build_tailbands(k2):
    """k2: [N, 8, 8] -> block-diag tail bands [N//GS, GS*21, 8, GS*14]."""
    n = k2.shape[0]
    tb = np.zeros((n // GS, GS * TK, KW, GS * TM), BF16)
    m = np.arange(TM)
    k2 = k2.astype(BF16)
    for g in range(n // GS):
        for j in range(GS):
            for p in range(KH):
                tb[g, TK * j + m + p, :, TM * j + m] = k2[g * GS + j, p, :]
    return tb


def kernel(x, k):
    x = np.asarray(x, dtype=np.float32).reshape(B, H, W)
    k = np.asarray(k, dtype=np.float32).reshape(B, KH, KW)

    if "runner" not in _cache:
        _cache["runner"] = _build_runner()
    _nc, run = _cache["runner"]

    xb = x.astype(BF16)
    bands = _build_bands(k)
    tailbands = _build_tailbands(k)
    n_groups = SPC // GS
    in_maps = [
        {
            "x": np.ascontiguousarray(xb[c * SPC : (c + 1) * SPC]),
            "bands": bands[c * SPC : (c + 1) * SPC],
            "tailbands": tailbands[c * n_groups : (c + 1) * n_groups],
        }
        for c in range(N_CORES)
    ]
    results = run(in_maps)
    out = np.concatenate([r["out"] for r in results], axis=0)
    return out.astype(np.float32).reshape(B, HO, WO, 1)
